# revision 1
# baseline (speedup 1.0000x reference)
"""AxialAttention Trainium2 Bass kernel.

Problem: q,k,v of shape (4, 8, 16, 32, 32, 64) = (b, heads, t, h, w, d),
attention along the h axis (axis 3), softmax over keys, out same shape.

Decomposition: the computation is 512 independent "slabs" (b, heads, t),
each a batch of w=32 independent length-32 attention problems with head
dim 64.  We shard 64 slabs per NeuronCore (8 cores), and process slabs in
"quads" (4 slabs = 128 partitions).

Per quad:
  - Load Q,K natural tiles [128=(s,h), 2048=(w,d)] with fp32->bf16 cast DMA.
  - DVE 32x32 stream-transpose -> QT,KT [128=(s,dlo), (w,db,h)].
  - Scores: per (w, db, s) a K=32 matmul at tile_position (32s, 0),
    accumulating db=0,1; outputs scores^T [k, q] in per-s PSUM banks.
  - exp on ScalarE (scale = 1/sqrt(64)) -> E_s bf16.
  - PV: per (w, s) a K=32 matmul lhsT=E block, rhs=[V | 1] (V augmented
    with a ones column so the softmax denominator falls out of the same
    matmul), tile_position (0, 32s) -> psum [(s,q), (w4, 65)].
  - reciprocal of denominators, copy unnormalized out, one broadcasted
    tensor_mul to normalize, store [128=(s,h), (w,d)] contiguous.
"""

import os
import sys
import numpy as np

for _p in ("/root/.axon_site/_ro/trn_rl_repo", "/opt/trn_rl_repo"):
    if os.path.isdir(_p) and _p not in sys.path:
        sys.path.append(_p)

B, NH, T, H, W, D = 4, 8, 16, 32, 32, 64
N_CORES = 8
NSLAB = B * NH * T  # 512
NSLAB_CORE = NSLAB // N_CORES  # 64
NQUAD = NSLAB_CORE // 4  # 16
VST = 80  # padded per-(s,w) V row: 64 d + 1 one + 15 pad (32B aligned)

_CACHED_NC = None


def _build_nc(n_slabs):
    import concourse.bacc as bacc
    import concourse.mybir as mybir
    from concourse import tile

    dt = mybir.dt
    nq = n_slabs // 4

    nc = bacc.Bacc("TRN2", target_bir_lowering=False, debug=False,
                   num_devices=N_CORES)
    q_in = nc.dram_tensor("q_in", [n_slabs, H, W, D], dt.bfloat16,
                          kind="ExternalInput").ap()
    k_in = nc.dram_tensor("k_in", [n_slabs, H, W, D], dt.bfloat16,
                          kind="ExternalInput").ap()
    v_in = nc.dram_tensor("v_in", [n_slabs, H, W, D], dt.bfloat16,
                          kind="ExternalInput").ap()
    o_out = nc.dram_tensor("o_out", [n_slabs, H, W, D], dt.float32,
                           kind="ExternalOutput").ap()

    scale = 1.0 / float(np.sqrt(D))

    with tile.TileContext(nc) as tc:
        with tc.tile_pool(name="io", bufs=3) as io_pool, \
             tc.tile_pool(name="tp", bufs=3) as tp_pool, \
             tc.tile_pool(name="vv", bufs=2) as v_pool, \
             tc.tile_pool(name="ee", bufs=3) as e_pool, \
             tc.tile_pool(name="oo", bufs=2) as o_pool, \
             tc.tile_pool(name="rr", bufs=2) as r_pool, \
             tc.tile_pool(name="ps_sc", bufs=1, space="PSUM") as ps_sc, \
             tc.tile_pool(name="ps_pv", bufs=1, space="PSUM") as ps_pv:

            quad_state = {}

            def emit_loads(g):
                s0 = 4 * g
                Q4 = io_pool.tile([128, W * D], dt.bfloat16, name="Q4")
                K4 = io_pool.tile([128, W * D], dt.bfloat16, name="K4")
                V4 = v_pool.tile([32, 4, W, VST], dt.bfloat16, name="V4")
                nc.sync.dma_start(
                    out=Q4[:, :],
                    in_=q_in[s0:s0 + 4].rearrange("s h w d -> (s h) (w d)"))
                nc.sync.dma_start(
                    out=K4[:, :],
                    in_=k_in[s0:s0 + 4].rearrange("s h w d -> (s h) (w d)"))
                for s in range(4):
                    nc.gpsimd.dma_start(
                        out=V4[:, s, :, 0:D],
                        in_=v_in[s0 + s])
                nc.vector.memset(V4[:, :, :, D:D + 1], 1.0)
                QT = tp_pool.tile([128, W * D], dt.bfloat16, name="QT")
                KT = tp_pool.tile([128, W * D], dt.bfloat16, name="KT")
                nc.vector.transpose(QT[:, :], Q4[:, :])
                nc.vector.transpose(KT[:, :], K4[:, :])
                out_sb = o_pool.tile([128, W, D], dt.float32, name="out_sb")
                R = r_pool.tile([128, W], dt.float32, name="R")
                quad_state[g] = dict(QT=QT, KT=KT, V4=V4, out_sb=out_sb, R=R)

            def emit_scores(g, chunk):
                qs = quad_state[g]
                QT, KT = qs["QT"], qs["KT"]
                w0 = 16 * chunk
                psS = [ps_sc.tile([32, 512], dt.float32, name=f"psS{s}")
                       for s in range(4)]
                Es = []
                # s-outer: each s-tile finishes early so its exp overlaps
                # the next s-tile's matmuls.
                for s in range(4):
                    for wl in range(16):
                        w = w0 + wl
                        for db in range(2):
                            c = (2 * w + db) * 32
                            nc.tensor.matmul(
                                psS[s][0:32, 32 * wl:32 * wl + 32],
                                lhsT=KT[32 * s:32 * s + 32, c:c + 32],
                                rhs=QT[32 * s:32 * s + 32, c:c + 32],
                                start=(db == 0), stop=(db == 1),
                                tile_position=(32 * s, 0))
                    E = e_pool.tile([32, 512], dt.bfloat16, name=f"E{s}")
                    nc.scalar.activation(
                        E[:, :], psS[s][:, :],
                        mybir.ActivationFunctionType.Exp, scale=scale)
                    Es.append(E)
                return Es

            def emit_pv(g, chunk, Es):
                qs = quad_state[g]
                V4, out_sb, R = qs["V4"], qs["out_sb"], qs["R"]
                w0 = 16 * chunk
                psPVs = [ps_pv.tile([128, 4, D + 1], dt.float32,
                                    name=f"psPV{i_}") for i_ in range(4)]
                for s in range(4):
                    for wl in range(16):
                        w = w0 + wl
                        psPV = psPVs[wl // 4]
                        wl4 = wl % 4
                        nc.tensor.matmul(
                            psPV[32 * s:32 * s + 32, wl4:wl4 + 1, 0:D + 1],
                            lhsT=Es[s][:, 32 * wl:32 * wl + 32],
                            rhs=V4[0:32, s, w, 0:D + 1],
                            start=True, stop=True,
                            tile_position=(0, 32 * s))
                for grp in range(4):
                    psPV = psPVs[grp]
                    nc.vector.reciprocal(
                        R[:, w0 + 4 * grp:w0 + 4 * grp + 4],
                        psPV[:, :, D])
                    nc.scalar.activation(
                        out_sb[:, w0 + 4 * grp:w0 + 4 * grp + 4, :],
                        psPV[:, :, 0:D],
                        mybir.ActivationFunctionType.Copy)

            def emit_finish(g):
                qs = quad_state.pop(g)
                out_sb, R = qs["out_sb"], qs["R"]
                s0 = 4 * g
                nc.vector.tensor_mul(
                    out_sb[:, :, :], out_sb[:, :, :],
                    R[:, :, None].broadcast_to([128, W, D]))
                nc.sync.dma_start(
                    out=o_out[s0:s0 + 4].rearrange("s h w d -> (s h) w d"),
                    in_=out_sb[:, :, :])

            # Software pipeline: PV of chunk t is emitted after the scores
            # of chunk t+1, so the PE queue always has runnable matmuls
            # while exp/copy of the previous chunk drain on ScalarE.
            emit_loads(0)
            pending = None  # (g, chunk, Es)
            for t in range(2 * nq):
                g, chunk = divmod(t, 2)
                if chunk == 0 and g + 1 < nq:
                    emit_loads(g + 1)
                Es = emit_scores(g, chunk)
                if pending is not None:
                    pg, pc, pEs = pending
                    emit_pv(pg, pc, pEs)
                    if pc == 1:
                        emit_finish(pg)
                pending = (g, chunk, Es)
            pg, pc, pEs = pending
            emit_pv(pg, pc, pEs)
            emit_finish(pg)
    nc.compile()
    return nc


def _get_nc():
    global _CACHED_NC
    if _CACHED_NC is None:
        _CACHED_NC = _build_nc(NSLAB_CORE)
    return _CACHED_NC


def kernel(q, k, v, decode_step=0, decode_idx=0, _trace=False):
    from concourse.bass_utils import run_bass_kernel_spmd

    import ml_dtypes
    bf16 = ml_dtypes.bfloat16
    q = np.asarray(q, dtype=np.float32).reshape(NSLAB, H, W, D).astype(bf16)
    k = np.asarray(k, dtype=np.float32).reshape(NSLAB, H, W, D).astype(bf16)
    v = np.asarray(v, dtype=np.float32).reshape(NSLAB, H, W, D).astype(bf16)

    nc = _get_nc()
    in_maps = []
    for c in range(N_CORES):
        sl = slice(c * NSLAB_CORE, (c + 1) * NSLAB_CORE)
        in_maps.append({
            "q_in": np.ascontiguousarray(q[sl]),
            "k_in": np.ascontiguousarray(k[sl]),
            "v_in": np.ascontiguousarray(v[sl]),
        })
    res = run_bass_kernel_spmd(nc, in_maps, core_ids=list(range(N_CORES)),
                               trace=_trace)
    out = np.concatenate([r["o_out"] for r in res.results], axis=0)
    out = out.reshape(B, NH, T, H, W, D)
    if _trace:
        return out, res
    return out


if __name__ == "__main__":
    rng = np.random.default_rng(0)
    shape = (B, NH, T, H, W, D)
    q = rng.standard_normal(shape, dtype=np.float32)
    k = rng.standard_normal(shape, dtype=np.float32)
    v = rng.standard_normal(shape, dtype=np.float32)
    out = kernel(q, k, v)
    print("kernel ran, out shape", out.shape)



# revision 2
# speedup vs baseline: 1.4331x; 1.4331x over previous
"""AxialAttention Trainium2 Bass kernel (batched-LDWEIGHTS rewrite).

Problem: q,k,v of shape (4, 8, 16, 32, 32, 64) = (b, heads, t, h, w, d),
attention along the h axis (axis 3), softmax over keys, out same shape.

512 independent slabs (b, heads, t), each a batch of w=32 length-32
attention problems with head dim 64.  64 slabs per core, processed in
quads (4 slabs).  Within a quad, slab s = 2*sl + sh: sl picks the
64-partition row half of QT/KT, sh picks the free-dim half.

Key ideas vs the naive version:
  - Host pre-lays-out Q/K transposed ([quad, (sl,d), (sh,w,h)]) and V
    augmented with a ones column ([quad, (om,h), (s,half,g,66)]), so every
    DMA is a single fully-contiguous 128-partition transfer and no
    on-chip transposes are needed.
  - Scores use K=64 contraction; ONE standalone 128x128 LDWEIGHTS feeds
    8 non-self-loading matmuls (2 row-pairs x 4 col groups).  PV: ONE
    strided-col LDWEIGHTS feeds 16 matmuls at 16 tile positions.
    Constraint: concurrent matmuls from different row groups must write
    different PSUM banks (bank per row group).
  - exp on full 128-partition [128,256] tiles; softmax denominator via
    the fused ones column; normalization as a DVE tensor_mul reading
    PSUM directly; output stored as bf16 and upcast on the host.
  - All tensor-engine instructions are chained with nosync deps so the
    tile scheduler preserves the LDW->matmul pairing.
"""

import os
import sys
import numpy as np

for _p in ("/root/.axon_site/_ro/trn_rl_repo", "/opt/trn_rl_repo"):
    if os.path.isdir(_p) and _p not in sys.path:
        sys.path.append(_p)

B, NH, T, H, W, D = 4, 8, 16, 32, 32, 64
N_CORES = 8
NSLAB = B * NH * T          # 512
NSLAB_CORE = NSLAB // N_CORES  # 64
NQ = NSLAB_CORE // 4        # 16 quads per core
NQ_ALL = NSLAB // 4         # 128 quads globally
VST = 66                    # 64 d + 1 one + 1 pad

_CACHED_NC = None


def _build_nc():
    import concourse.bacc as bacc
    import concourse.mybir as mybir
    from concourse import tile
    from concourse.tile_rust import add_dep_helper

    dt = mybir.dt

    nc = bacc.Bacc("TRN2", target_bir_lowering=False, debug=False,
                   num_devices=N_CORES)
    qt_in = nc.dram_tensor("qt_in", [NQ, 128, 2048], dt.bfloat16,
                           kind="ExternalInput").ap()
    kt_in = nc.dram_tensor("kt_in", [NQ, 128, 2048], dt.bfloat16,
                           kind="ExternalInput").ap()
    v_in = nc.dram_tensor("v_in", [NQ, 128, 4 * 2 * 4 * VST], dt.bfloat16,
                          kind="ExternalInput").ap()
    o_out = nc.dram_tensor("o_out", [NQ, 128, 2048], dt.bfloat16,
                           kind="ExternalOutput").ap()

    scale = 1.0 / float(np.sqrt(D))

    with tile.TileContext(nc) as tc:
        with tc.tile_pool(name="io", bufs=4) as io_pool, \
             tc.tile_pool(name="ee", bufs=2) as e_pool, \
             tc.tile_pool(name="rr", bufs=2) as r_pool, \
             tc.tile_pool(name="oo", bufs=2) as o_pool, \
             tc.tile_pool(name="ps_sc", bufs=2, space="PSUM") as ps_sc, \
             tc.tile_pool(name="ps_pv", bufs=1, space="PSUM") as ps_pv:

            chain = [None]

            def tchain(bi):
                inst = bi.ins if hasattr(bi, "ins") else bi
                if chain[0] is not None:
                    add_dep_helper(inst, chain[0], sync=False,
                                   reason="pe order")
                chain[0] = inst
                return bi

            qstate = {}

            def emit_loads(qi):
                QT = io_pool.tile([128, 2, W, H], dt.bfloat16, name="QT")
                KT = io_pool.tile([128, 2, W, H], dt.bfloat16, name="KT")
                V = io_pool.tile([128, 4, 2, 4, VST], dt.bfloat16, name="V")
                nc.sync.dma_start(
                    out=QT[:, :, :, :],
                    in_=qt_in[qi].rearrange("p (a w h) -> p a w h",
                                            a=2, w=W))
                nc.sync.dma_start(
                    out=KT[:, :, :, :],
                    in_=kt_in[qi].rearrange("p (a w h) -> p a w h",
                                            a=2, w=W))
                nc.scalar.dma_start(
                    out=V[:, :, :, :, :],
                    in_=v_in[qi].rearrange("p (s f g x) -> p s f g x",
                                           s=4, f=2, g=4))
                OUT = o_pool.tile([128, 2, 4, 4, D], dt.bfloat16, name="OUT")
                R = r_pool.tile([128, 2, 4, 4], dt.float32, name="R")
                qstate[qi] = dict(QT=QT, KT=KT, V=V, OUT=OUT, R=R)

            def emit_scores(qi, half):
                st = qstate[qi]
                QT, KT = st["QT"], st["KT"]
                psc = [ps_sc.tile([128, 2, 4, H], dt.float32,
                                  name=f"psc{sl}") for sl in range(2)]
                E = e_pool.tile([128, 4, 4, H], dt.bfloat16, name="E")
                w0 = 16 * half
                for sh in range(2):
                    for g in range(4):
                        wb = w0 + 4 * g
                        tchain(nc.tensor.ldweights(KT[:, sh, wb:wb + 4, :]))
                        for sl in range(2):
                            for om in range(4):
                                w = wb + om
                                mm = nc.tensor.matmul(
                                    psc[sl][32 * om:32 * om + 32, sh, g, :],
                                    lhsT=KT[64 * sl:64 * sl + 64, sh, w, :],
                                    rhs=QT[64 * sl:64 * sl + 64, sh, w, :],
                                    start=True, stop=True,
                                    tile_position=(64 * sl, 32 * om))
                                mm.ins.ldweights = False
                                tchain(mm)
                for sl in range(2):
                    nc.scalar.activation(
                        E[:, 2 * sl:2 * sl + 2, :, :],
                        psc[sl][:, :, :, :],
                        mybir.ActivationFunctionType.Exp, scale=scale)
                return E

            def emit_pv(qi, half, E):
                st = qstate[qi]
                V, OUT, R = st["V"], st["OUT"], st["R"]
                psv = [ps_pv.tile([128, 4, D + 1], dt.float32,
                                  name=f"psv{om}") for om in range(4)]
                for g in range(4):
                    tchain(nc.tensor.ldweights(E[:, :, g, :]))
                    for om in range(4):
                        for s in range(4):
                            mm = nc.tensor.matmul(
                                psv[om][32 * s:32 * s + 32, g, 0:D + 1],
                                lhsT=E[32 * om:32 * om + 32, s, g, :],
                                rhs=V[32 * om:32 * om + 32, s, half, g,
                                      0:D + 1],
                                start=True, stop=True,
                                tile_position=(32 * om, 32 * s))
                            mm.ins.ldweights = False
                            tchain(mm)
                for om in range(4):
                    nc.vector.reciprocal(R[:, half, :, om],
                                         psv[om][:, :, D])
                    nc.vector.tensor_mul(
                        OUT[:, half, :, om, :],
                        psv[om][:, :, 0:D],
                        R[:, half, :, om, None].broadcast_to([128, 4, D]))

            def emit_finish(qi):
                st = qstate.pop(qi)
                nc.scalar.dma_start(out=o_out[qi],
                                    in_=st["OUT"][:, :, :, :, :])

            emit_loads(0)
            pending = None
            for t in range(2 * NQ):
                qi, half = divmod(t, 2)
                if half == 0 and qi + 1 < NQ:
                    emit_loads(qi + 1)
                E = emit_scores(qi, half)
                if pending is not None:
                    pqi, phalf, pE = pending
                    emit_pv(pqi, phalf, pE)
                    if phalf == 1:
                        emit_finish(pqi)
                pending = (qi, half, E)
            pqi, phalf, pE = pending
            emit_pv(pqi, phalf, pE)
            emit_finish(pqi)
    nc.compile()
    return nc


def _get_nc():
    global _CACHED_NC
    if _CACHED_NC is None:
        _CACHED_NC = _build_nc()
    return _CACHED_NC


def kernel(q, k, v, decode_step=0, decode_idx=0, _trace=False):
    from concourse.bass_utils import run_bass_kernel_spmd
    import ml_dtypes

    bf16 = ml_dtypes.bfloat16

    q = np.asarray(q, dtype=np.float32).reshape(NSLAB, H, W, D)
    k = np.asarray(k, dtype=np.float32).reshape(NSLAB, H, W, D)
    v = np.asarray(v, dtype=np.float32).reshape(NSLAB, H, W, D)

    # QT/KT: [quad, (sl, d)=128, (sh, w, h)=2048]
    qg = q.reshape(NQ_ALL, 2, 2, H, W, D).astype(bf16)
    qt = np.ascontiguousarray(qg.transpose(0, 1, 5, 2, 4, 3)) \
        .reshape(NQ_ALL, 128, 2048)
    kg = k.reshape(NQ_ALL, 2, 2, H, W, D).astype(bf16)
    kt = np.ascontiguousarray(kg.transpose(0, 1, 5, 2, 4, 3)) \
        .reshape(NQ_ALL, 128, 2048)
    # V: [quad, (om, h)=128, (s, half, g, VST)]; x=64 is the ones column
    vg = v.reshape(NQ_ALL, 4, H, 2, 4, 4, D).astype(bf16)
    v_aug = np.empty((NQ_ALL, 4, H, 4, 2, 4, VST), dtype=bf16)
    v_aug[..., :D] = vg.transpose(0, 5, 2, 1, 3, 4, 6)
    v_aug[..., D] = 1.0
    v_aug[..., D + 1:] = 0.0
    v_pre = v_aug.reshape(NQ_ALL, 128, 4 * 2 * 4 * VST)

    nc = _get_nc()
    in_maps = []
    for c in range(N_CORES):
        sl = slice(c * NQ, (c + 1) * NQ)
        in_maps.append({
            "qt_in": qt[sl],
            "kt_in": kt[sl],
            "v_in": v_pre[sl],
        })
    res = run_bass_kernel_spmd(nc, in_maps, core_ids=list(range(N_CORES)),
                               trace=_trace)
    o = np.concatenate([r["o_out"] for r in res.results], axis=0)
    # o: [quad, (s, q)=128, (half, g, om, d)=2048] == [slab, h, w, d]
    out = o.reshape(NSLAB, H, W, D).astype(np.float32)
    out = out.reshape(B, NH, T, H, W, D)
    if _trace:
        return out, res
    return out


if __name__ == "__main__":
    rng = np.random.default_rng(0)
    shape = (B, NH, T, H, W, D)
    q = rng.standard_normal(shape, dtype=np.float32)
    k = rng.standard_normal(shape, dtype=np.float32)
    v = rng.standard_normal(shape, dtype=np.float32)
    out = kernel(q, k, v)
    print("kernel ran, out shape", out.shape)
